# revision 28
# baseline (speedup 1.0000x reference)
"""Trainium2 Bass kernel for CRF forward-algorithm loss (logsumexp scan).

v3: meet-in-the-middle sequence parallelism + wider matmuls.

Math (exp domain): u_t = d_t * (P @ u_{t-1}), P = exp(trans), d = exp(emit-C).
loss_lane = log(pstop . u_511) = log(m . (P @ u_255)) where m is the CRF
backward vector: m = d_256 * (P^T (d_257 * (P^T ... (d_511 * pstop)))).

Sharding: cores 0-3 run the forward scan over emissions 0..255 (32 lanes
each); cores 4-7 run the backward scan over emissions 511..256 for the same
lane blocks. Both sides are the identical program (matmul-then-multiply
recurrence) with different weights (P^T vs P), init column (P[:,START] vs
pstop) and emission order (ascending vs descending). The host applies the
middle P matmul in f64 and combines. 255 matmul-steps per core instead of
512, with N=32 matmuls instead of 2x N=8.

No renormalization: with C=7.23 the per-lane scale drift stays within a few
e-folds over 256 steps (measured), so bf16/f32 range is never stressed; the
host bookkeeping only needs the final vectors.

Per step: 16 LDW+MM pairs (4 out-chunks x 4 contract-chunks, N=32), ordered
contract-chunks {0,1} first then {2,3} so the next step's first matmuls
depend only on the first DVE multiply; psum and u are split into two tiles
(chunks 01 / 23) to keep dependency tracking fine-grained.
"""

import numpy as np
import ml_dtypes

import concourse.bass as bass
import concourse.mybir as mybir
import concourse.tile as tile
from concourse import bacc
from concourse.bass_utils import run_bass_kernel_spmd

T = 512
S = 512
B = 128
NCORES = 8
NSIDE = 4          # cores per side (fwd / bwd)
L = B // NSIDE     # 32 lanes per core
TC = 4             # tag chunks of 128
M = 256            # steps per side (step 0 = init mul, 1..255 = mm+mul)
DG = 16            # steps per DMA group
NG = M // DG       # 16 groups
START = 510
STOP = 511
C = 7.23

F32 = mybir.dt.float32
BF16 = mybir.dt.bfloat16

bf = ml_dtypes.bfloat16


def _build_program():
    nc = bacc.Bacc(
        "TRN2",
        target_bir_lowering=False,
        debug=False,
        enable_asserts=False,
        num_devices=NCORES,
    )

    pt_d = nc.dram_tensor("pt", [128, TC * TC * 128], BF16, kind="ExternalInput")
    ucol_d = nc.dram_tensor("ucol", [128, TC * L], BF16, kind="ExternalInput")
    em_d = nc.dram_tensor("emt", [NG, 128, DG * TC * L], BF16,
                          kind="ExternalInput")
    fin_d = nc.dram_tensor("fin", [128, TC * L], BF16, kind="ExternalOutput")

    H = TC * L // 2   # 64 cols per half (chunks 01 / 23)

    with tile.TileContext(nc) as tc:
        with (
            tc.tile_pool(name="singles", bufs=1) as singles,
            tc.tile_pool(name="empool", bufs=6) as empool,
            tc.tile_pool(name="ehpool", bufs=6) as ehpool,
            tc.tile_pool(name="uabpool", bufs=3) as uabpool,
            tc.tile_pool(name="ucdpool", bufs=3) as ucdpool,
            tc.tile_pool(name="psabpool", bufs=2, space="PSUM") as psabpool,
            tc.tile_pool(name="pscdpool", bufs=2, space="PSUM") as pscdpool,
        ):
            # em group 0 first: the t=0 chain (dma -> exp -> mul) gates the
            # first matmul; only a head slice (first 4 steps) is needed to
            # start, the tail streams behind ucol/pt.
            HD = 4 * TC * L
            em0 = empool.tile([128, DG * TC * L], BF16, name="em8", tag="em")
            nc.sync.dma_start(out=em0[:, 0:HD], in_=em_d[0][:, 0:HD])
            ucol_sb = singles.tile([128, TC * L], BF16)
            nc.sync.dma_start(out=ucol_sb, in_=ucol_d[:, :])
            ptsb = singles.tile([128, TC * TC * 128], BF16)
            nc.sync.dma_start(out=ptsb, in_=pt_d[:, :])
            nc.sync.dma_start(out=em0[:, HD:], in_=em_d[0][:, HD:])
            negc_sb = singles.tile([128, 1], F32)
            nc.vector.memset(negc_sb, -C)

            def w(i, j):
                return ptsb[:, (i * TC + j) * 128 : (i * TC + j + 1) * 128]

            uAB = uCD = None
            eh = None
            for t in range(M):
                s = t % DG
                if s == 0:
                    g = t // DG
                    if g == 0:
                        em8 = em0
                    else:
                        em8 = empool.tile([128, DG * TC * L], BF16, name="em8",
                                          tag="em")
                        nc.sync.dma_start(out=em8, in_=em_d[g])
                    eh = ehpool.tile([128, DG * TC * L], BF16, name="eh8",
                                     tag="eh")
                    if g == 0:
                        # split the first exp so step 0's slice is ready early
                        nc.scalar.activation(
                            eh[:, 0 : TC * L], em8[:, 0 : TC * L],
                            mybir.ActivationFunctionType.Exp,
                            bias=negc_sb, scale=1.0,
                        )
                        nc.scalar.activation(
                            eh[:, TC * L :], em8[:, TC * L :],
                            mybir.ActivationFunctionType.Exp,
                            bias=negc_sb, scale=1.0,
                        )
                    else:
                        nc.scalar.activation(
                            eh, em8, mybir.ActivationFunctionType.Exp,
                            bias=negc_sb, scale=1.0,
                        )
                ehAB = eh[:, s * TC * L : s * TC * L + H]
                ehCD = eh[:, s * TC * L + H : (s + 1) * TC * L]

                uAB_new = uabpool.tile([128, H], BF16, name="uAB", tag="uAB")
                uCD_new = ucdpool.tile([128, H], BF16, name="uCD", tag="uCD")

                if t == 0:
                    nc.vector.tensor_mul(uAB_new, ucol_sb[:, 0:H], ehAB)
                    nc.vector.tensor_mul(uCD_new, ucol_sb[:, H : 2 * H], ehCD)
                else:
                    psAB = psabpool.tile([128, H], F32, name="psAB", tag="pa")
                    psCD = pscdpool.tile([128, H], F32, name="psCD", tag="pc")

                    def ps(j):
                        return (psAB[:, j * L : (j + 1) * L] if j < 2
                                else psCD[:, (j - 2) * L : (j - 1) * L])

                    def uc(i):
                        return (uAB[:, i * L : (i + 1) * L] if i < 2
                                else uCD[:, (i - 2) * L : (i - 1) * L])

                    # Slot order: chunks {0,1} first (so next step's early
                    # matmuls need only the first DVE mul), with psAB's
                    # regions (0,1) finishing at slot 9 so mul01 starts early;
                    # psCD's chunk-23 work fills the tail.
                    # start=True clears has_written for the WHOLE bank, so it
                    # may only appear on the first matmul touching each psum
                    # tile this step; later writes to a fresh region overwrite
                    # (bit clear) then accumulate, which is exactly right.
                    slots = [(0, 0), (0, 1), (1, 0), (1, 1), (2, 0), (2, 1),
                             (0, 2), (0, 3), (1, 2), (1, 3), (3, 0), (3, 1),
                             (2, 2), (2, 3), (3, 2), (3, 3)]
                    for j, i in slots:
                        nc.tensor.matmul(
                            ps(j), w(i, j), uc(i),
                            start=(i == 0 and j in (0, 2)), stop=(i == 3),
                            skip_group_check=True,
                        )
                    nc.vector.tensor_mul(uAB_new, psAB, ehAB)
                    nc.vector.tensor_mul(uCD_new, psCD, ehCD)

                uAB, uCD = uAB_new, uCD_new

            nc.sync.dma_start(out=fin_d[:, 0:H], in_=uAB)
            nc.sync.dma_start(out=fin_d[:, H : 2 * H], in_=uCD)

    nc.compile()
    return nc


def _prep_inputs(emissions, transitions):
    P = np.exp(transitions.astype(np.float64)).astype(np.float32)

    def tiles_for(A):
        # lhsT tile (i, j) = A[i*128:(i+1)*128, j*128:(j+1)*128]
        return np.ascontiguousarray(
            A.reshape(TC, 128, TC, 128).transpose(1, 0, 2, 3)
        ).reshape(128, TC * TC * 128).astype(bf)

    pt_f = tiles_for(np.ascontiguousarray(P.T))   # fwd: lhsT = P^T
    pt_b = tiles_for(P)                           # bwd: lhsT = P

    def colrep(v):
        # [T] -> [128, TC*L] laid out (chunk, lane), replicated across lanes
        return np.ascontiguousarray(
            np.repeat(v.reshape(TC, 128).T[:, :, None], L, axis=2)
        ).reshape(128, TC * L).astype(bf)

    ucol_f = colrep(P[:, START])
    ucol_b = colrep(P[STOP, :])

    em_bf = emissions.astype(bf)   # [B, S, T]

    def emt_for(lane0, steps):
        sh = em_bf[lane0 : lane0 + L][:, steps, :]       # [L, M, T]
        a = sh.reshape(L, NG, DG, TC, 128)               # [b, g, s, i, p]
        return np.ascontiguousarray(
            a.transpose(1, 4, 2, 3, 0)                   # [g, p, s, i, b]
        ).reshape(NG, 128, DG * TC * L)

    steps_f = np.arange(M)
    steps_b = np.arange(S - 1, M - 1, -1)

    in_maps = []
    for c in range(NSIDE):
        in_maps.append({"pt": pt_f, "ucol": ucol_f,
                        "emt": emt_for(c * L, steps_f)})
    for c in range(NSIDE):
        in_maps.append({"pt": pt_b, "ucol": ucol_b,
                        "emt": emt_for(c * L, steps_b)})
    return in_maps


def _loss_from_outputs(results, transitions):
    P64 = np.exp(transitions.astype(np.float64))
    total = 0.0
    for c in range(NSIDE):
        # fin layout [128, (chunk, lane)] -> [T, L]
        def vec(res):
            a = np.asarray(res["fin"]).astype(np.float64)
            return a.reshape(128, TC, L).transpose(1, 0, 2).reshape(T, L)

        u = vec(results[c])
        m = vec(results[c + NSIDE])
        y = P64 @ u
        dot = (m * y).sum(axis=0)
        total += (np.log(dot) + S * C).sum()
    return np.float32(total)


def _run(inputs, **kwargs):
    emissions = np.asarray(inputs["inputs"], dtype=np.float32)
    transitions = np.asarray(inputs["transitions"], dtype=np.float32)
    assert emissions.shape == (B, S, T), emissions.shape
    nc = _build_program()
    in_maps = _prep_inputs(emissions, transitions)
    res = run_bass_kernel_spmd(nc, in_maps, core_ids=list(range(NCORES)), **kwargs)
    return _loss_from_outputs(res.results, transitions), res


def kernel(**inputs) -> np.ndarray:
    out, _ = _run(inputs)
    return out
